# revision 46
# baseline (speedup 1.0000x reference)
"""Multi-head attention (B=2, S=2048, D=1024, H=16) on 8 TRN2 NeuronCores.

Sharding: core c handles batch c//4 and head-group c%4 (4 heads each).
Host pre-transposes inputs/weights to d-major bf16; each core computes its
4 heads' projections, causal attention, and a partial (row-parallel) dense
output [S, D] which the host sums across the 4 cores of each batch.

Attention math: scores are computed transposed ([k, q] layout, q on the
free dim) so no on-chip transposes are ever needed.  Causal masking is
applied INSIDE the scores psum accumulation: an extra matmul with an
identity lhsT adds -65536 to masked elements (exact for 0/1 masks), so the
vector engine is off the scores->exp->av critical path.  Softmax row sums
come for free from a ones column in the AV lhsT (position 64 for even
heads, position 0 for odd heads, whose AV lands directly on partitions
64:128 -- no partition-shift DMA).  Reciprocals are partition-broadcast
through a DRAM bounce mid-kernel (latency absorbed downstream) and via a
tiny bf16 selector matmul on the final, latency-critical chain.  The PE
clock ramps with sustained use (0.65 -> 1.2 -> 2.4 GHz over ~3us), so
warm-up matmuls run during the initial input DMA and all dense-projection
work is interleaved into the scalar-exp-heavy attention windows as filler
to keep the PE from ever idling and re-throttling.
"""

import numpy as np
import ml_dtypes
from contextlib import ExitStack

import concourse.bass as bass
import concourse.tile as tile
from concourse import bacc, mybir
from concourse.bass_utils import run_bass_kernel_spmd

BF16 = mybir.dt.bfloat16
F32 = mybir.dt.float32
NPBF16 = ml_dtypes.bfloat16

D_MODEL = 1024
NH = 16
DEPTH = 64
B = 2
S = 2048
N_CORES = 8
GROUPS = 4              # head-groups (tensor parallel dimension)
HPG = NH // GROUPS      # 4 heads per core
OG = HPG * DEPTH        # 256 projection output cols per core
QC = 512                # q chunk (matmul free dim)
NQC = S // QC           # 4
KT = 128                # k tile (psum partition dim)
NKT = S // KT           # 16
DK = D_MODEL // 128     # 8 contraction tiles of 128
SC = 512                # projection s chunk
NSC = S // SC           # 4
EGRP = 2                # k-tiles per exp group (psum group tile)
NEG = -65536.0          # additive mask value (exact in bf16)
NWARM512 = 16           # warm-up matmuls of 512 cols
NWARM128 = 24           # warm-up matmuls of 128 cols

TRACE = False
TRACE_KW = {}
LAST_RESULT = None
_CACHE = {}


def _chunk(lst, n):
    return [lst[i : i + n] for i in range(0, len(lst), n)]


def _build(ktiles, n_uniq, zero_bias, binary):
    """Emit the bass program. ktiles[j] = [(t, lo, tri), ...] computed
    k-tiles for q-chunk j (see _classify_mask); binary[uid] says whether
    factor tile uid is 0/1-valued (-> additive mask matmul)."""
    nc = bacc.Bacc(
        "TRN2", target_bir_lowering=False, debug=False, num_devices=N_CORES
    )
    # inputs pre-split into contiguous S-quarters for fat DMA descriptors
    xq = nc.dram_tensor("xq", [NSC, 128, DK, SC], BF16, kind="ExternalInput").ap()
    xk = nc.dram_tensor("xk", [NSC, 128, DK, SC], BF16, kind="ExternalInput").ap()
    xv = nc.dram_tensor("xv", [NSC, 128, DK, SC], BF16, kind="ExternalInput").ap()
    wq = nc.dram_tensor("wq", [128, DK, OG], BF16, kind="ExternalInput").ap()
    wk = nc.dram_tensor("wk", [128, DK, OG], BF16, kind="ExternalInput").ap()
    wv = nc.dram_tensor("wv", [128, DK, OG], BF16, kind="ExternalInput").ap()
    wd = nc.dram_tensor("wd", [128, 2, D_MODEL], BF16, kind="ExternalInput").ap()
    qb = nc.dram_tensor("qb", [128, 2], F32, kind="ExternalInput").ap()
    kb = nc.dram_tensor("kb", [128, 2], F32, kind="ExternalInput").ap()
    mk = nc.dram_tensor("mk", [128, n_uniq, KT], BF16, kind="ExternalInput").ap()
    am = nc.dram_tensor("am", [128, n_uniq, KT], BF16, kind="ExternalInput").ap()
    ident = nc.dram_tensor("ident", [128, 128], BF16, kind="ExternalInput").ap()
    eb = nc.dram_tensor("eb", [128, 2, 128], BF16, kind="ExternalInput").ap()
    outp = nc.dram_tensor("outp", [S, D_MODEL], BF16, kind="ExternalOutput").ap()

    Exp = mybir.ActivationFunctionType.Exp

    with tile.TileContext(nc) as tc, ExitStack() as ctx:
        singles = ctx.enter_context(tc.tile_pool(name="singles", bufs=1))
        exps = ctx.enter_context(tc.tile_pool(name="exps", bufs=3))
        small = ctx.enter_context(tc.tile_pool(name="small", bufs=3))
        bcastp = ctx.enter_context(tc.tile_pool(name="bcastp", bufs=4))
        dram = ctx.enter_context(tc.tile_pool(name="dram", bufs=3, space="DRAM"))
        # psum: 2 banks proj/av/dense rotation + 6 banks scores groups
        ppav = ctx.enter_context(tc.tile_pool(name="ppav", bufs=2, space="PSUM"))
        psc = ctx.enter_context(tc.tile_pool(name="psc", bufs=3, space="PSUM"))
        ost = ctx.enter_context(tc.tile_pool(name="ost", bufs=3))

        # ---- warm-up: keep the PE busy (and ramping) during input DMA ----
        warm_sb = singles.tile([128, 512], BF16)
        nc.gpsimd.memset(warm_sb[:], 1.0)
        warm_ps = psc.tile([128, EGRP, QC], F32, tag="psc")
        for i in range(NWARM512):
            nc.tensor.matmul(
                warm_ps[:, 0, :],
                lhsT=warm_sb[:, 0:128],
                rhs=warm_sb[:],
                start=True,
                stop=True,
                skip_group_check=True,
            )
        for i in range(NWARM128):
            nc.tensor.matmul(
                warm_ps[:, 0, 0:128],
                lhsT=warm_sb[:, 0:128],
                rhs=warm_sb[:, 0:128],
                start=True,
                stop=True,
                skip_group_check=True,
            )

        wq_sb = singles.tile([128, DK, OG], BF16)
        wk_sb = singles.tile([128, DK, OG], BF16)
        wv_sb = singles.tile([128, DK, OG], BF16)
        mk_sb = singles.tile([128, n_uniq, KT], BF16)
        am_sb = singles.tile([128, n_uniq, KT], BF16)
        id_sb = singles.tile([128, 128], BF16)
        eb_sb = singles.tile([128, 2, 128], BF16)
        recbfe = singles.tile([128, QC], BF16)
        recbfo = singles.tile([128, QC], BF16)
        qb_sb = singles.tile([128, 2], F32)
        kb_sb = singles.tile([128, 2], F32)
        wd_sb = singles.tile([128, 2, D_MODEL], BF16)  # loaded late, before dense

        # q: packed layout [p, ch, s]; head h = 2*ch + (p>=64), d = p%64.
        # k: per-head zero-padded layout so each scores lhsT isolates a head
        # (the padding kills qt's packed sibling in the contraction).
        qt = singles.tile([128, 2, S], BF16)
        kt_ = singles.tile([128, HPG, S], BF16)
        # even heads: [64 d cols + ones col] -> av on psum 0:64, den 64
        # odd heads:  [ones, zeros, 64 d] -> av on psum 64:128, den 0
        # (partition 0: directly reciprocal-able, no partition bounce)
        # (odd av lands on its home partitions; no shift DMA needed)
        vh1e = singles.tile([128, NKT, 2, 65], BF16)
        vh1o = singles.tile([128, NKT, 2, 128], BF16)
        avf = singles.tile([128, 2, S], F32)    # unnormalized av^T
        avb = singles.tile([128, 2, S], BF16)   # normalized av^T

        nc.gpsimd.memset(recbfe[:], 0.0)
        nc.gpsimd.memset(recbfo[:], 0.0)
        nc.gpsimd.memset(vh1e[:, :, :, 64:65], 1.0)
        nc.gpsimd.memset(vh1o[:, :, :, 0:64], 0.0)
        nc.gpsimd.memset(vh1o[:, :, :, 0:1], 1.0)
        ktv = kt_.rearrange("p (a b) s -> p a b s", b=2)
        nc.gpsimd.memset(ktv[64:128, :, 0, :], 0.0)
        nc.gpsimd.memset(ktv[0:64, :, 1, :], 0.0)

        def emit_proj_chunk(xin, sc, mid_cb=None):
            """Project q/k/v for s-chunk sc (the data attention j=sc needs).
            mid_cb (if given) is emitted after the q section -- used to place
            the previous chunk's deferred ch1 den chain where its bounce DMA
            has already landed."""
            ssl = slice(sc * SC, (sc + 1) * SC)
            first = sc == 0
            # q projection: packed destination, one copy per oc
            x_sb = xin.tile([128, DK, SC], BF16, tag="xin")
            if first:
                # chunk-0 loads split in two so the first projection matmuls
                # can start on the first half; weight loads interleaved in
                # consumption order
                nc.sync.dma_start(x_sb[:, 0:4, :], xq[sc][:, 0:4, :])
                nc.sync.dma_start(x_sb[:, 4:8, :], xq[sc][:, 4:8, :])
                nc.sync.dma_start(wk_sb[:], wk)
            else:
                nc.sync.dma_start(x_sb[:], xq[sc])
            for oc in range(2):
                ps = ppav.tile([128, SC], F32, tag="ppav")
                for dk in range(DK):
                    nc.tensor.matmul(
                        ps[:],
                        lhsT=wq_sb[:, dk, oc * 128 : (oc + 1) * 128],
                        rhs=x_sb[:, dk, :],
                        start=(dk == 0),
                        stop=(dk == DK - 1),
                    )
                if zero_bias:
                    nc.vector.tensor_copy(out=qt[:, oc, ssl], in_=ps[:])
                else:
                    nc.vector.tensor_scalar(
                        out=qt[:, oc, ssl],
                        in0=ps[:],
                        scalar1=qb_sb[:, oc : oc + 1],
                        scalar2=None,
                        op0=mybir.AluOpType.add,
                    )
            if mid_cb is not None:
                mid_cb()
            # k projection: padded per-head destination, two half copies
            x_sb = xin.tile([128, DK, SC], BF16, tag="xin")
            if first:
                nc.sync.dma_start(x_sb[:, 0:4, :], xk[sc][:, 0:4, :])
                nc.sync.dma_start(x_sb[:, 4:8, :], xk[sc][:, 4:8, :])
                nc.sync.dma_start(wv_sb[:], wv)
            else:
                nc.sync.dma_start(x_sb[:], xk[sc])
            for oc in range(2):
                ps = ppav.tile([128, SC], F32, tag="ppav")
                for dk in range(DK):
                    nc.tensor.matmul(
                        ps[:],
                        lhsT=wk_sb[:, dk, oc * 128 : (oc + 1) * 128],
                        rhs=x_sb[:, dk, :],
                        start=(dk == 0),
                        stop=(dk == DK - 1),
                    )
                if zero_bias:
                    nc.vector.tensor_copy(
                        out=kt_[0:64, 2 * oc, ssl], in_=ps[0:64, :]
                    )
                    nc.vector.tensor_copy(
                        out=kt_[64:128, 2 * oc + 1, ssl], in_=ps[64:128, :]
                    )
                else:
                    nc.vector.tensor_scalar(
                        out=kt_[0:64, 2 * oc, ssl],
                        in0=ps[0:64, :],
                        scalar1=kb_sb[0:64, oc : oc + 1],
                        scalar2=None,
                        op0=mybir.AluOpType.add,
                    )
                    nc.vector.tensor_scalar(
                        out=kt_[64:128, 2 * oc + 1, ssl],
                        in0=ps[64:128, :],
                        scalar1=kb_sb[64:128, oc : oc + 1],
                        scalar2=None,
                        op0=mybir.AluOpType.add,
                    )
            xv_sb = xin.tile([128, DK, SC], BF16, tag="xin")
            nc.sync.dma_start(xv_sb[:], xv[sc])
            if first:
                nc.sync.dma_start(id_sb[:], ident)
                nc.sync.dma_start(am_sb[:], am)
                nc.sync.dma_start(eb_sb[:], eb)
            for sth in range(SC // KT):
                st = sc * (SC // KT) + sth
                ps = ppav.tile([128, SC], F32, tag="ppav")
                for dk in range(DK):
                    nc.tensor.matmul(
                        ps[:, :OG],
                        lhsT=xv_sb[:, dk, sth * KT : (sth + 1) * KT],
                        rhs=wv_sb[:, dk, :],
                        start=(dk == 0),
                        stop=(dk == DK - 1),
                    )
                psv = ps[:, :OG].rearrange(
                    "p (g two d) -> p g two d", two=2, d=DEPTH
                )
                nc.vector.tensor_copy(
                    out=vh1e[:, st, :, 0:64], in_=psv[:, :, 0, :]
                )
                nc.vector.tensor_copy(
                    out=vh1o[:, st, :, 64:128], in_=psv[:, :, 1, :]
                )

        def emit_attn(h, j, dch, skip_dma=False):
            odd = h % 2
            ch = h // 2
            tiles = ktiles[j]
            first, last = tiles[0][0], tiles[-1][0]
            ps_av = ppav.tile([128, QC], F32, tag="ppav")
            groups = _chunk(tiles, EGRP)
            for grp in groups:
                lo_min = min(g[1] for g in grp)
                ps_g = psc.tile([128, EGRP, QC], F32, tag="psc")
                for r, (t, lo, tri) in enumerate(grp):
                    # cols [0, lo*128) are fully masked: never computed,
                    # never read by the av matmul below
                    bin_tri = [x for x in tri if binary[x[1]]]
                    nc.tensor.matmul(
                        ps_g[:, r, lo * 128 :],
                        lhsT=kt_[:, h, t * KT : (t + 1) * KT],
                        rhs=qt[:, ch, j * QC + lo * 128 : (j + 1) * QC],
                        start=True,
                        stop=(not bin_tri),
                        skip_group_check=True,
                    )
                    # additive causal mask folded into the psum accumulation
                    for n, (i, uid) in enumerate(bin_tri):
                        nc.tensor.matmul(
                            ps_g[:, r, i * 128 : (i + 1) * 128],
                            lhsT=id_sb[:],
                            rhs=am_sb[:, uid, :],
                            start=False,
                            stop=(n == len(bin_tri) - 1),
                            skip_group_check=True,
                        )
                ex = exps.tile([128, EGRP, QC], BF16, tag="exps")
                nc.scalar.activation(
                    out=ex[:, : len(grp), lo_min * 128 :],
                    in_=ps_g[:, : len(grp), lo_min * 128 :],
                    func=Exp,
                    scale=0.125,
                )
                for r, (t, lo, tri) in enumerate(grp):
                    for i, uid in tri:
                        if not binary[uid]:
                            nc.vector.tensor_mul(
                                ex[:, r, i * 128 : (i + 1) * 128],
                                ex[:, r, i * 128 : (i + 1) * 128],
                                mk_sb[:, uid, :],
                            )
                for r, (t, lo, tri) in enumerate(grp):
                    if odd:
                        nc.tensor.matmul(
                            ps_av[:, lo * 128 :],
                            lhsT=vh1o[:, t, ch, :],
                            rhs=ex[:, r, lo * 128 :],
                            start=(t == first),
                            stop=(t == last),
                        )
                    else:
                        nc.tensor.matmul(
                            ps_av[0:65, lo * 128 :],
                            lhsT=vh1e[:, t, ch, :],
                            rhs=ex[:, r, lo * 128 :],
                            start=(t == first),
                            stop=(t == last),
                        )
            # denominator staging first (it gates the normalize multiply):
            # psum den row -> sbuf -> partition 0/1 of the pair tile
            dcp = small.tile([128, QC], F32, tag="dcp")
            dp = 0 if odd else 64
            nc.vector.tensor_copy(out=dcp[dp : dp + 1, :], in_=ps_av[dp : dp + 1, :])
            if not odd:
                nc.sync.dma_start(dch[0:1, :], dcp[dp : dp + 1, :])
            elif not skip_dma:
                nc.sync.dma_start(dch[1:2, :], dcp[dp : dp + 1, :])
            # stage unnormalized av into sbuf; both parities land on their
            # home partitions, no shift needed
            if odd:
                nc.vector.tensor_copy(
                    out=avf[64:128, ch, j * QC : (j + 1) * QC],
                    in_=ps_av[64:128, :],
                )
            else:
                nc.vector.tensor_copy(
                    out=avf[0:64, ch, j * QC : (j + 1) * QC],
                    in_=ps_av[0:64, :],
                )
            return dcp

        def emit_den_chain(ch, j, dch, odd_dcp=None, fast=False):
            """After heads (2ch, 2ch+1) of chunk j: reciprocal of the two
            staged denominator rows, partition-broadcast, normalize. The
            final chain (fast=True) broadcasts via a tiny bf16 selector
            matmul; mid-kernel chains bounce through DRAM, staying off the
            psum ring entirely (their latency is absorbed downstream)."""
            if fast:
                # even half: recip its bounced den row (landed during the
                # odd head's attention); odd half: recip directly from the
                # odd ps_av partition-0 den copy (no DMA in the tail path)
                rce = small.tile([1, QC], F32, tag="rce")
                nc.vector.reciprocal_approx_fast(rce[:], dch[0:1, :])
                nc.vector.tensor_copy(out=recbfe[0:1, :], in_=rce[:])
                rco = small.tile([1, QC], F32, tag="rco")
                nc.vector.reciprocal_approx_fast(rco[:], odd_dcp[0:1, :])
                nc.vector.tensor_copy(out=recbfo[0:1, :], in_=rco[:])
                bct = ppav.tile([128, QC], F32, tag="ppav")
                nc.tensor.matmul(
                    bct[:], lhsT=eb_sb[:, 0, :], rhs=recbfe[:],
                    start=True, stop=False, skip_group_check=True,
                )
                nc.tensor.matmul(
                    bct[:], lhsT=eb_sb[:, 1, :], rhs=recbfo[:],
                    start=False, stop=True, skip_group_check=True,
                )
                nc.vector.tensor_mul(
                    avb[:, ch, j * QC : (j + 1) * QC],
                    avf[:, ch, j * QC : (j + 1) * QC],
                    bct[:],
                )
                return
            rec = small.tile([2, QC], F32, tag="rec")
            nc.vector.reciprocal_approx_fast(rec[:], dch[:])
            rdr = dram.tile([2, QC], F32, tag="rdr")
            nc.sync.dma_start(rdr[:], rec[:])
            bc = bcastp.tile([128, QC], F32, tag="bc")
            nc.sync.dma_start(bc[0:64, :], rdr[0:1, :].to_broadcast([64, QC]))
            nc.sync.dma_start(bc[64:128, :], rdr[1:2, :].to_broadcast([64, QC]))
            # all-SBUF multiply runs on the otherwise-idle gpsimd engine so
            # the in-order vector queue never stalls waiting for the bc DMA
            nc.gpsimd.tensor_mul(
                avb[:, ch, j * QC : (j + 1) * QC],
                avf[:, ch, j * QC : (j + 1) * QC],
                bc[:],
            )

        def emit_dense_st(st, tail=False):
            ot = ost.tile([128, D_MODEL], BF16, tag="ostage")
            for oc in range(2):
                ps = ppav.tile([128, SC], F32, tag="ppav")
                for co in range(2):
                    nc.tensor.matmul(
                        ps[:],
                        lhsT=avb[:, co, st * 128 : (st + 1) * 128],
                        rhs=wd_sb[:, co, oc * 512 : (oc + 1) * 512],
                        start=(co == 0),
                        stop=(co == 1),
                    )
                if tail and oc == 0:
                    # post-exp tail: scalar is idle, parallelize staging
                    nc.scalar.copy(
                        out=ot[:, oc * 512 : (oc + 1) * 512], in_=ps[:]
                    )
                else:
                    nc.vector.tensor_copy(
                        out=ot[:, oc * 512 : (oc + 1) * 512], in_=ps[:]
                    )
            # latency-tolerant output writes dispatch from gpsimd's
            # software DGE, keeping the sync queue free for x/den DMAs
            nc.gpsimd.dma_start(outp[st * 128 : (st + 1) * 128, :], ot[:])

        # ---- interleaved emission: proj chunk sc -> attention j=sc; all
        # dense blocks are deferred into the (scalar-exp-heavy) j=3 window
        # as tensor filler so the PE never idles while exp drains ----
        with tc.tile_pool(name="xin", bufs=3) as xin:
            # wq first; wk/wv/aux issue inside emit_proj_chunk(0) in
            # consumption order (each dma_start costs ~600ns of sync-engine
            # issue time, so order matters more than splitting)
            nc.sync.dma_start(wq_sb[:], wq)
            if any(not b for b in binary):
                nc.sync.dma_start(mk_sb[:], mk)
            if not zero_bias:
                nc.sync.dma_start(qb_sb[:], qb)
                nc.sync.dma_start(kb_sb[:], kb)
            pend1 = None
            for sc in range(NSC):
                emit_proj_chunk(xin, sc, mid_cb=pend1)
                pend1 = None
                if sc == 0:
                    nc.sync.dma_start(wd_sb[:], wd)  # dense-weight prefetch
                last = sc == NSC - 1
                pend = None
                for h in range(HPG):
                    if h % 2 == 0:
                        dch = small.tile([2, QC], F32, tag="dch")
                    fast_h = last and h == HPG - 1
                    cur_dcp = emit_attn(h, sc, dch, skip_dma=fast_h)
                    if h == 2 and pend is not None:
                        # ch0's chain was deferred past h2's attention so its
                        # reciprocal never stalls the vector queue waiting on
                        # the den bounce DMA
                        emit_den_chain(0, sc, pend)
                        pend = None
                    if sc in (1, 2):
                        emit_dense_st(4 * (sc - 1) + h)
                    elif last and h < HPG - 1:
                        emit_dense_st(8 + h)
                        if h == HPG - 2:
                            emit_dense_st(11)
                    if h % 2 == 1:
                        fast = last and h == HPG - 1
                        if h == 1:
                            pend = dch
                            continue
                        if fast:
                            # keep the PE p-state up while the final
                            # reciprocal chain drains
                            wp = psc.tile([128, EGRP, QC], F32, tag="psc")
                            for _ in range(10):
                                nc.tensor.matmul(
                                    wp[:, 0, :],
                                    lhsT=warm_sb[:, 0:128],
                                    rhs=warm_sb[:],
                                    start=True,
                                    stop=True,
                                    skip_group_check=True,
                                )
                        odd_dcp = cur_dcp
                        if not last and h == HPG - 1:
                            pend1 = (
                                lambda ch=h // 2, j=sc, d=dch:
                                emit_den_chain(ch, j, d)
                            )
                        else:
                            emit_den_chain(
                                h // 2, sc, dch, odd_dcp=odd_dcp, fast=fast
                            )
                    if last and h == HPG - 1:
                        for st in (12, 13, 14, 15):
                            emit_dense_st(st, tail=True)

    nc.compile()
    return nc


def _classify_mask(mask):
    """Classify 128(k) x 128(q) score blocks from the actual mask contents.

    Returns (ktiles, mk_arr, binary):
      ktiles[j]: list of (t, lo, tri) per computed k-tile for q-chunk j:
        lo: first kept 128-col block within the 512-wide q-chunk (cols
            [0, lo*128) are fully masked and simply never computed/read)
        tri: [(col_block, uid), ...] 128-col blocks needing masking
      mk_arr: [128, NU, 128] bf16 multiplicative factors exp(-1e9*m/8)
      binary: per-uid flag, True when the factor tile is 0/1-valued
    """
    m2 = np.asarray(mask, dtype=np.float32).reshape(S, S)
    F = np.exp(m2 * np.float32(-1.25e8))  # exp(-1e9*m/8); 0/1 masks -> 0/1
    if (F.max(axis=1) == 0.0).any():
        raise RuntimeError("mask has fully-masked rows; unsupported")
    blocks = F.reshape(NKT, 128, NKT, 128)  # [qi, qr, t, kr]
    kept = (blocks == 1.0).all(axis=(1, 3))  # [qi, t]
    skip = (blocks == 0.0).all(axis=(1, 3))

    NB = QC // 128  # 128-col blocks per q-chunk
    ktiles = []
    uniq = {}
    mk_tiles = []

    def factor_uid(qi, t):
        fb = np.ascontiguousarray(
            F[qi * 128 : (qi + 1) * 128, t * KT : (t + 1) * KT].T
        ).astype(NPBF16)
        key = fb.tobytes()
        if key not in uniq:
            uniq[key] = len(mk_tiles)
            mk_tiles.append(fb)
        return uniq[key]

    for j in range(NQC):
        qis = list(range(j * NB, (j + 1) * NB))
        tl = []
        for t in range(NKT):
            stats = [
                "k" if kept[qi, t] else ("s" if skip[qi, t] else "m")
                for qi in qis
            ]
            if all(s == "s" for s in stats):
                continue
            lo = next(i for i, s in enumerate(stats) if s != "s")
            tri = []
            for i in range(lo, NB):
                if stats[i] == "k":
                    continue
                # mixed OR interior skip (multiply by its factor / zeros)
                tri.append((i, factor_uid(qis[i], t)))
            tl.append((t, lo, tri))
        if not tl:
            raise RuntimeError("q-chunk with no kept k-tiles; unsupported")
        # the first computed tile must span the full chunk (av 'start' MM)
        if tl[0][1] != 0:
            t0, _, tri0 = tl[0]
            tri0 = [(i, u) for i, u in tri0]
            have = {i for i, _ in tri0}
            for i in range(tl[0][1]):
                if i not in have:
                    tri0.append((i, factor_uid(qis[i], t0)))
            tl[0] = (t0, 0, sorted(tri0))
        ktiles.append(tl)
    if not mk_tiles:
        mk_tiles.append(np.ones((128, KT), dtype=NPBF16))
    binary = tuple(
        bool(np.isin(t.astype(np.float32), [0.0, 1.0]).all()) for t in mk_tiles
    )
    mk_arr = np.ascontiguousarray(np.stack(mk_tiles, axis=0).transpose(1, 0, 2))
    # additive variant: NEG where factor==0, 0 where factor==1
    am_arr = np.where(
        mk_arr.astype(np.float32) == 0.0, np.float32(NEG), np.float32(0.0)
    ).astype(NPBF16)
    return ktiles, mk_arr, am_arr, binary


def _xt_prep(x):
    """[S, D] f32 -> [NSC, 128, DK, SC] bf16, d-major, contiguous S-quarters."""
    xt = x.T.astype(NPBF16)  # [D, S]
    a = xt.reshape(DK, 128, NSC, SC).transpose(2, 1, 0, 3)
    return np.ascontiguousarray(a)


def kernel(v, k, q, mask, wq_w, wq_b, wk_w, wk_b, wv_w, wv_b, dense_w, dense_b):
    global LAST_RESULT
    v = np.asarray(v, dtype=np.float32)
    k = np.asarray(k, dtype=np.float32)
    q = np.asarray(q, dtype=np.float32)
    mask = np.asarray(mask, dtype=np.float32)
    wq_w = np.asarray(wq_w, dtype=np.float32)
    wk_w = np.asarray(wk_w, dtype=np.float32)
    wv_w = np.asarray(wv_w, dtype=np.float32)
    dense_w = np.asarray(dense_w, dtype=np.float32)
    wq_b = np.asarray(wq_b, dtype=np.float32)
    wk_b = np.asarray(wk_b, dtype=np.float32)
    wv_b = np.asarray(wv_b, dtype=np.float32)
    dense_b = np.asarray(dense_b, dtype=np.float32)

    ktiles, mk_arr, am_arr, binary = _classify_mask(mask)
    zero_bias = not (np.any(wq_b) or np.any(wk_b))
    key = (
        tuple(tuple((t, lo, tuple(tri)) for t, lo, tri in tl) for tl in ktiles),
        mk_arr.shape[1],
        zero_bias,
        binary,
    )
    if key not in _CACHE:
        _CACHE[key] = _build(ktiles, mk_arr.shape[1], zero_bias, binary)
    nc = _CACHE[key]

    # per-batch inputs (shared by the 4 cores of each batch)
    xq_b = [_xt_prep(q[b]) for b in range(B)]
    xk_b = [_xt_prep(k[b]) for b in range(B)]
    xv_b = [_xt_prep(v[b]) for b in range(B)]

    # per-group weights
    def wslice(w, g):
        ws = w[g * OG : (g + 1) * OG, :].T.astype(NPBF16)  # [D, OG]
        return np.ascontiguousarray(ws.reshape(DK, 128, OG).transpose(1, 0, 2))

    def bslice(b_, g):
        return np.ascontiguousarray(
            b_[g * OG : (g + 1) * OG].astype(np.float32).reshape(2, 128).T
        )

    wq_g = [wslice(wq_w, g) for g in range(GROUPS)]
    wk_g = [wslice(wk_w, g) for g in range(GROUPS)]
    wv_g = [wslice(wv_w, g) for g in range(GROUPS)]
    qb_g = [bslice(wq_b, g) for g in range(GROUPS)]
    kb_g = [bslice(wk_b, g) for g in range(GROUPS)]
    wd_g = []
    for g in range(GROUPS):
        ds = dense_w[:, g * OG : (g + 1) * OG].T.astype(NPBF16)  # [OG, D]
        wd_g.append(np.ascontiguousarray(ds.reshape(2, 128, D_MODEL).transpose(1, 0, 2)))

    id_arr = np.eye(128, dtype=NPBF16)
    # split broadcast selectors (both read reciprocal row 0 of their rhs):
    # slot 0 covers even-head partitions 0:64, slot 1 odd partitions 64:128
    eb_arr = np.zeros((128, 2, 128), dtype=NPBF16)
    eb_arr[0, 0, 0:64] = 1.0
    eb_arr[0, 1, 64:128] = 1.0

    in_maps = []
    for c in range(N_CORES):
        b, g = c // GROUPS, c % GROUPS
        in_maps.append(
            {
                "xq": xq_b[b],
                "xk": xk_b[b],
                "xv": xv_b[b],
                "wq": wq_g[g],
                "wk": wk_g[g],
                "wv": wv_g[g],
                "wd": wd_g[g],
                "qb": qb_g[g],
                "kb": kb_g[g],
                "mk": mk_arr,
                "am": am_arr,
                "ident": id_arr,
                "eb": eb_arr,
            }
        )

    kw = dict(trace=True, **TRACE_KW) if TRACE else {}
    res = run_bass_kernel_spmd(nc, in_maps, core_ids=list(range(N_CORES)), **kw)
    LAST_RESULT = res

    corr = dense_w @ wv_b + dense_b  # v-bias pushed through dense, + dense bias
    out = np.empty((B, S, D_MODEL), dtype=np.float32)
    for b in range(B):
        acc = np.zeros((S, D_MODEL), dtype=np.float32)
        for g in range(GROUPS):
            acc += res.results[b * GROUPS + g]["outp"]
        out[b] = acc + corr
    return out


# revision 47
# speedup vs baseline: 1.0216x; 1.0216x over previous
"""Multi-head attention (B=2, S=2048, D=1024, H=16) on 8 TRN2 NeuronCores.

Sharding: core c handles batch c//4 and head-group c%4 (4 heads each).
Host pre-transposes inputs/weights to d-major bf16; each core computes its
4 heads' projections, causal attention, and a partial (row-parallel) dense
output [S, D] which the host sums across the 4 cores of each batch.

Attention math: scores are computed transposed ([k, q] layout, q on the
free dim) so no on-chip transposes are ever needed.  Causal masking is
applied INSIDE the scores psum accumulation: an extra matmul with an
identity lhsT adds -65536 to masked elements (exact for 0/1 masks), so the
vector engine is off the scores->exp->av critical path.  Softmax row sums
come for free from a ones column in the AV lhsT (position 64 for even
heads, position 0 for odd heads, whose AV lands directly on partitions
64:128 -- no partition-shift DMA).  Reciprocals are partition-broadcast
through a DRAM bounce mid-kernel (latency absorbed downstream) and via a
tiny bf16 selector matmul on the final, latency-critical chain.  The PE
clock ramps with sustained use (0.65 -> 1.2 -> 2.4 GHz over ~3us), so
warm-up matmuls run during the initial input DMA and all dense-projection
work is interleaved into the scalar-exp-heavy attention windows as filler
to keep the PE from ever idling and re-throttling.
"""

import numpy as np
import ml_dtypes
from contextlib import ExitStack

import concourse.bass as bass
import concourse.tile as tile
from concourse import bacc, mybir
from concourse.bass_utils import run_bass_kernel_spmd

BF16 = mybir.dt.bfloat16
F32 = mybir.dt.float32
NPBF16 = ml_dtypes.bfloat16

D_MODEL = 1024
NH = 16
DEPTH = 64
B = 2
S = 2048
N_CORES = 8
GROUPS = 4              # head-groups (tensor parallel dimension)
HPG = NH // GROUPS      # 4 heads per core
OG = HPG * DEPTH        # 256 projection output cols per core
QC = 512                # q chunk (matmul free dim)
NQC = S // QC           # 4
KT = 128                # k tile (psum partition dim)
NKT = S // KT           # 16
DK = D_MODEL // 128     # 8 contraction tiles of 128
SC = 512                # projection s chunk
NSC = S // SC           # 4
EGRP = 2                # k-tiles per exp group (psum group tile)
NEG = -65536.0          # additive mask value (exact in bf16)
NWARM512 = 16           # warm-up matmuls of 512 cols
NWARM128 = 24           # warm-up matmuls of 128 cols

TRACE = False
TRACE_KW = {}
LAST_RESULT = None
_CACHE = {}


def _chunk(lst, n):
    return [lst[i : i + n] for i in range(0, len(lst), n)]


def _build(ktiles, n_uniq, zero_bias, binary):
    """Emit the bass program. ktiles[j] = [(t, lo, tri), ...] computed
    k-tiles for q-chunk j (see _classify_mask); binary[uid] says whether
    factor tile uid is 0/1-valued (-> additive mask matmul)."""
    nc = bacc.Bacc(
        "TRN2", target_bir_lowering=False, debug=False, num_devices=N_CORES
    )
    # inputs pre-split into contiguous S-quarters for fat DMA descriptors
    xq = nc.dram_tensor("xq", [NSC, 128, DK, SC], BF16, kind="ExternalInput").ap()
    xk = nc.dram_tensor("xk", [NSC, 128, DK, SC], BF16, kind="ExternalInput").ap()
    xv = nc.dram_tensor("xv", [NSC, 128, DK, SC], BF16, kind="ExternalInput").ap()
    wq = nc.dram_tensor("wq", [128, DK, OG], BF16, kind="ExternalInput").ap()
    wk = nc.dram_tensor("wk", [128, DK, OG], BF16, kind="ExternalInput").ap()
    wv = nc.dram_tensor("wv", [128, DK, OG], BF16, kind="ExternalInput").ap()
    wd = nc.dram_tensor("wd", [128, 2, D_MODEL], BF16, kind="ExternalInput").ap()
    qb = nc.dram_tensor("qb", [128, 2], F32, kind="ExternalInput").ap()
    kb = nc.dram_tensor("kb", [128, 2], F32, kind="ExternalInput").ap()
    mk = nc.dram_tensor("mk", [128, n_uniq, KT], BF16, kind="ExternalInput").ap()
    am = nc.dram_tensor("am", [128, n_uniq, KT], BF16, kind="ExternalInput").ap()
    ident = nc.dram_tensor("ident", [128, 128], BF16, kind="ExternalInput").ap()
    eb = nc.dram_tensor("eb", [128, 2, 128], BF16, kind="ExternalInput").ap()
    outp = nc.dram_tensor("outp", [S, D_MODEL], BF16, kind="ExternalOutput").ap()

    Exp = mybir.ActivationFunctionType.Exp

    with tile.TileContext(nc) as tc, ExitStack() as ctx:
        singles = ctx.enter_context(tc.tile_pool(name="singles", bufs=1))
        exps = ctx.enter_context(tc.tile_pool(name="exps", bufs=3))
        small = ctx.enter_context(tc.tile_pool(name="small", bufs=3))
        bcastp = ctx.enter_context(tc.tile_pool(name="bcastp", bufs=4))
        dram = ctx.enter_context(tc.tile_pool(name="dram", bufs=3, space="DRAM"))
        # psum: 2 banks proj/av/dense rotation + 6 banks scores groups
        ppav = ctx.enter_context(tc.tile_pool(name="ppav", bufs=2, space="PSUM"))
        psc = ctx.enter_context(tc.tile_pool(name="psc", bufs=3, space="PSUM"))
        ost = ctx.enter_context(tc.tile_pool(name="ost", bufs=3))

        # ---- warm-up: keep the PE busy (and ramping) during input DMA ----
        warm_sb = singles.tile([128, 512], BF16)
        nc.gpsimd.memset(warm_sb[:], 1.0)
        warm_ps = psc.tile([128, EGRP, QC], F32, tag="psc")
        for i in range(NWARM512):
            nc.tensor.matmul(
                warm_ps[:, 0, :],
                lhsT=warm_sb[:, 0:128],
                rhs=warm_sb[:],
                start=True,
                stop=True,
                skip_group_check=True,
            )
        for i in range(NWARM128):
            nc.tensor.matmul(
                warm_ps[:, 0, 0:128],
                lhsT=warm_sb[:, 0:128],
                rhs=warm_sb[:, 0:128],
                start=True,
                stop=True,
                skip_group_check=True,
            )

        wq_sb = singles.tile([128, DK, OG], BF16)
        wk_sb = singles.tile([128, DK, OG], BF16)
        wv_sb = singles.tile([128, DK, OG], BF16)
        mk_sb = singles.tile([128, n_uniq, KT], BF16)
        am_sb = singles.tile([128, n_uniq, KT], BF16)
        id_sb = singles.tile([128, 128], BF16)
        eb_sb = singles.tile([128, 2, 128], BF16)
        recbfe = singles.tile([128, QC], BF16)
        recbfo = singles.tile([128, QC], BF16)
        qb_sb = singles.tile([128, 2], F32)
        kb_sb = singles.tile([128, 2], F32)
        wd_sb = singles.tile([128, 2, D_MODEL], BF16)  # loaded late, before dense

        # q: packed layout [p, ch, s]; head h = 2*ch + (p>=64), d = p%64.
        # k: per-head zero-padded layout so each scores lhsT isolates a head
        # (the padding kills qt's packed sibling in the contraction).
        qt = singles.tile([128, 2, S], BF16)
        kt_ = singles.tile([128, HPG, S], BF16)
        # even heads: [64 d cols + ones col] -> av on psum 0:64, den 64
        # odd heads:  [ones, zeros, 64 d] -> av on psum 64:128, den 0
        # (partition 0: directly reciprocal-able, no partition bounce)
        # (odd av lands on its home partitions; no shift DMA needed)
        vh1e = singles.tile([128, NKT, 2, 65], BF16)
        vh1o = singles.tile([128, NKT, 2, 128], BF16)
        avf = singles.tile([128, 2, S], F32)    # unnormalized av^T
        avb = singles.tile([128, 2, S], BF16)   # normalized av^T

        nc.gpsimd.memset(recbfe[:], 0.0)
        nc.gpsimd.memset(recbfo[:], 0.0)
        nc.gpsimd.memset(vh1e[:, :, :, 64:65], 1.0)
        nc.gpsimd.memset(vh1o[:, :, :, 0:64], 0.0)
        nc.gpsimd.memset(vh1o[:, :, :, 0:1], 1.0)
        ktv = kt_.rearrange("p (a b) s -> p a b s", b=2)
        nc.gpsimd.memset(ktv[64:128, :, 0, :], 0.0)
        nc.gpsimd.memset(ktv[0:64, :, 1, :], 0.0)

        def emit_proj_chunk(xin, sc, mid_cb=None):
            """Project q/k/v for s-chunk sc (the data attention j=sc needs).
            mid_cb (if given) is emitted after the q section -- used to place
            the previous chunk's deferred ch1 den chain where its bounce DMA
            has already landed."""
            ssl = slice(sc * SC, (sc + 1) * SC)
            first = sc == 0
            # q projection: packed destination, one copy per oc
            x_sb = xin.tile([128, DK, SC], BF16, tag="xin")
            if first:
                # chunk-0 loads split in two so the first projection matmuls
                # can start on the first half; weight loads interleaved in
                # consumption order
                nc.sync.dma_start(x_sb[:, 0:4, :], xq[sc][:, 0:4, :])
                nc.sync.dma_start(x_sb[:, 4:8, :], xq[sc][:, 4:8, :])
                nc.sync.dma_start(wk_sb[:], wk)
            else:
                nc.sync.dma_start(x_sb[:], xq[sc])
            for oc in range(2):
                ps = ppav.tile([128, SC], F32, tag="ppav")
                for dk in range(DK):
                    nc.tensor.matmul(
                        ps[:],
                        lhsT=wq_sb[:, dk, oc * 128 : (oc + 1) * 128],
                        rhs=x_sb[:, dk, :],
                        start=(dk == 0),
                        stop=(dk == DK - 1),
                    )
                if zero_bias:
                    nc.vector.tensor_copy(out=qt[:, oc, ssl], in_=ps[:])
                else:
                    nc.vector.tensor_scalar(
                        out=qt[:, oc, ssl],
                        in0=ps[:],
                        scalar1=qb_sb[:, oc : oc + 1],
                        scalar2=None,
                        op0=mybir.AluOpType.add,
                    )
            if mid_cb is not None:
                mid_cb()
            # k projection: padded per-head destination, two half copies
            x_sb = xin.tile([128, DK, SC], BF16, tag="xin")
            if first:
                nc.sync.dma_start(x_sb[:, 0:4, :], xk[sc][:, 0:4, :])
                nc.sync.dma_start(x_sb[:, 4:8, :], xk[sc][:, 4:8, :])
                nc.sync.dma_start(wv_sb[:], wv)
            else:
                nc.sync.dma_start(x_sb[:], xk[sc])
            for oc in range(2):
                ps = ppav.tile([128, SC], F32, tag="ppav")
                for dk in range(DK):
                    nc.tensor.matmul(
                        ps[:],
                        lhsT=wk_sb[:, dk, oc * 128 : (oc + 1) * 128],
                        rhs=x_sb[:, dk, :],
                        start=(dk == 0),
                        stop=(dk == DK - 1),
                    )
                if zero_bias:
                    nc.vector.tensor_copy(
                        out=kt_[0:64, 2 * oc, ssl], in_=ps[0:64, :]
                    )
                    nc.vector.tensor_copy(
                        out=kt_[64:128, 2 * oc + 1, ssl], in_=ps[64:128, :]
                    )
                else:
                    nc.vector.tensor_scalar(
                        out=kt_[0:64, 2 * oc, ssl],
                        in0=ps[0:64, :],
                        scalar1=kb_sb[0:64, oc : oc + 1],
                        scalar2=None,
                        op0=mybir.AluOpType.add,
                    )
                    nc.vector.tensor_scalar(
                        out=kt_[64:128, 2 * oc + 1, ssl],
                        in0=ps[64:128, :],
                        scalar1=kb_sb[64:128, oc : oc + 1],
                        scalar2=None,
                        op0=mybir.AluOpType.add,
                    )
            xv_sb = xin.tile([128, DK, SC], BF16, tag="xin")
            nc.sync.dma_start(xv_sb[:], xv[sc])
            if first:
                nc.sync.dma_start(id_sb[:], ident)
                nc.sync.dma_start(am_sb[:], am)
                nc.sync.dma_start(eb_sb[:], eb)
            for sth in range(SC // KT):
                st = sc * (SC // KT) + sth
                ps = ppav.tile([128, SC], F32, tag="ppav")
                for dk in range(DK):
                    nc.tensor.matmul(
                        ps[:, :OG],
                        lhsT=xv_sb[:, dk, sth * KT : (sth + 1) * KT],
                        rhs=wv_sb[:, dk, :],
                        start=(dk == 0),
                        stop=(dk == DK - 1),
                    )
                psv = ps[:, :OG].rearrange(
                    "p (g two d) -> p g two d", two=2, d=DEPTH
                )
                nc.vector.tensor_copy(
                    out=vh1e[:, st, :, 0:64], in_=psv[:, :, 0, :]
                )
                nc.vector.tensor_copy(
                    out=vh1o[:, st, :, 64:128], in_=psv[:, :, 1, :]
                )

        def emit_attn(h, j, dch, skip_dma=False):
            odd = h % 2
            ch = h // 2
            tiles = ktiles[j]
            first, last = tiles[0][0], tiles[-1][0]
            ps_av = ppav.tile([128, QC], F32, tag="ppav")
            groups = _chunk(tiles, EGRP)
            for grp in groups:
                lo_min = min(g[1] for g in grp)
                ps_g = psc.tile([128, EGRP, QC], F32, tag="psc")
                for r, (t, lo, tri) in enumerate(grp):
                    # cols [0, lo*128) are fully masked: never computed,
                    # never read by the av matmul below
                    bin_tri = [x for x in tri if binary[x[1]]]
                    nc.tensor.matmul(
                        ps_g[:, r, lo * 128 :],
                        lhsT=kt_[:, h, t * KT : (t + 1) * KT],
                        rhs=qt[:, ch, j * QC + lo * 128 : (j + 1) * QC],
                        start=True,
                        stop=(not bin_tri),
                        skip_group_check=True,
                    )
                    # additive causal mask folded into the psum accumulation
                    for n, (i, uid) in enumerate(bin_tri):
                        nc.tensor.matmul(
                            ps_g[:, r, i * 128 : (i + 1) * 128],
                            lhsT=id_sb[:],
                            rhs=am_sb[:, uid, :],
                            start=False,
                            stop=(n == len(bin_tri) - 1),
                            skip_group_check=True,
                        )
                ex = exps.tile([128, EGRP, QC], BF16, tag="exps")
                nc.scalar.activation(
                    out=ex[:, : len(grp), lo_min * 128 :],
                    in_=ps_g[:, : len(grp), lo_min * 128 :],
                    func=Exp,
                    scale=0.125,
                )
                for r, (t, lo, tri) in enumerate(grp):
                    for i, uid in tri:
                        if not binary[uid]:
                            nc.vector.tensor_mul(
                                ex[:, r, i * 128 : (i + 1) * 128],
                                ex[:, r, i * 128 : (i + 1) * 128],
                                mk_sb[:, uid, :],
                            )
                for r, (t, lo, tri) in enumerate(grp):
                    if odd:
                        nc.tensor.matmul(
                            ps_av[:, lo * 128 :],
                            lhsT=vh1o[:, t, ch, :],
                            rhs=ex[:, r, lo * 128 :],
                            start=(t == first),
                            stop=(t == last),
                        )
                    else:
                        nc.tensor.matmul(
                            ps_av[0:65, lo * 128 :],
                            lhsT=vh1e[:, t, ch, :],
                            rhs=ex[:, r, lo * 128 :],
                            start=(t == first),
                            stop=(t == last),
                        )
            # denominator staging first (it gates the normalize multiply):
            # psum den row -> sbuf -> partition 0/1 of the pair tile
            dcp = small.tile([128, QC], F32, tag="dcp")
            dp = 0 if odd else 64
            nc.vector.tensor_copy(out=dcp[dp : dp + 1, :], in_=ps_av[dp : dp + 1, :])
            if not odd:
                nc.sync.dma_start(dch[0:1, :], dcp[dp : dp + 1, :])
            elif not skip_dma:
                nc.sync.dma_start(dch[1:2, :], dcp[dp : dp + 1, :])
            # stage unnormalized av into sbuf; both parities land on their
            # home partitions, no shift needed
            if odd:
                nc.vector.tensor_copy(
                    out=avf[64:128, ch, j * QC : (j + 1) * QC],
                    in_=ps_av[64:128, :],
                )
            else:
                nc.vector.tensor_copy(
                    out=avf[0:64, ch, j * QC : (j + 1) * QC],
                    in_=ps_av[0:64, :],
                )
            return dcp

        def emit_den_chain(ch, j, dch, odd_dcp=None, fast=False):
            """After heads (2ch, 2ch+1) of chunk j: reciprocal of the two
            staged denominator rows, partition-broadcast, normalize. The
            final chain (fast=True) broadcasts via a tiny bf16 selector
            matmul; mid-kernel chains bounce through DRAM, staying off the
            psum ring entirely (their latency is absorbed downstream)."""
            if fast:
                # even half: recip its bounced den row (landed during the
                # odd head's attention); odd half: recip directly from the
                # odd ps_av partition-0 den copy (no DMA in the tail path)
                rce = small.tile([1, QC], F32, tag="rce")
                nc.vector.reciprocal_approx_fast(rce[:], dch[0:1, :])
                nc.vector.tensor_copy(out=recbfe[0:1, :], in_=rce[:])
                rco = small.tile([1, QC], F32, tag="rco")
                nc.vector.reciprocal_approx_fast(rco[:], odd_dcp[0:1, :])
                nc.vector.tensor_copy(out=recbfo[0:1, :], in_=rco[:])
                bct = ppav.tile([128, QC], F32, tag="ppav")
                nc.tensor.matmul(
                    bct[:], lhsT=eb_sb[:, 0, :], rhs=recbfe[:],
                    start=True, stop=False, skip_group_check=True,
                )
                nc.tensor.matmul(
                    bct[:], lhsT=eb_sb[:, 1, :], rhs=recbfo[:],
                    start=False, stop=True, skip_group_check=True,
                )
                nc.vector.tensor_mul(
                    avb[:, ch, j * QC : (j + 1) * QC],
                    avf[:, ch, j * QC : (j + 1) * QC],
                    bct[:],
                )
                return
            rec = small.tile([2, QC], F32, tag="rec")
            nc.vector.reciprocal_approx_fast(rec[:], dch[:])
            rdr = dram.tile([2, QC], F32, tag="rdr")
            nc.sync.dma_start(rdr[:], rec[:])
            bc = bcastp.tile([128, QC], F32, tag="bc")
            nc.sync.dma_start(bc[0:64, :], rdr[0:1, :].to_broadcast([64, QC]))
            nc.sync.dma_start(bc[64:128, :], rdr[1:2, :].to_broadcast([64, QC]))
            # all-SBUF multiply runs on the otherwise-idle gpsimd engine so
            # the in-order vector queue never stalls waiting for the bc DMA
            nc.gpsimd.tensor_mul(
                avb[:, ch, j * QC : (j + 1) * QC],
                avf[:, ch, j * QC : (j + 1) * QC],
                bc[:],
            )

        def emit_dense_st(st, tail=False):
            ot = ost.tile([128, D_MODEL], BF16, tag="ostage")
            for oc in range(2):
                ps = ppav.tile([128, SC], F32, tag="ppav")
                for co in range(2):
                    nc.tensor.matmul(
                        ps[:],
                        lhsT=avb[:, co, st * 128 : (st + 1) * 128],
                        rhs=wd_sb[:, co, oc * 512 : (oc + 1) * 512],
                        start=(co == 0),
                        stop=(co == 1),
                    )
                if tail and oc == 0:
                    # post-exp tail: scalar is idle, parallelize staging
                    nc.scalar.copy(
                        out=ot[:, oc * 512 : (oc + 1) * 512], in_=ps[:]
                    )
                else:
                    nc.vector.tensor_copy(
                        out=ot[:, oc * 512 : (oc + 1) * 512], in_=ps[:]
                    )
            nc.sync.dma_start(outp[st * 128 : (st + 1) * 128, :], ot[:])

        # ---- interleaved emission: proj chunk sc -> attention j=sc; all
        # dense blocks are deferred into the (scalar-exp-heavy) j=3 window
        # as tensor filler so the PE never idles while exp drains ----
        with tc.tile_pool(name="xin", bufs=3) as xin:
            # wq first; wk/wv/aux issue inside emit_proj_chunk(0) in
            # consumption order (each dma_start costs ~600ns of sync-engine
            # issue time, so order matters more than splitting)
            nc.sync.dma_start(wq_sb[:], wq)
            if any(not b for b in binary):
                nc.sync.dma_start(mk_sb[:], mk)
            if not zero_bias:
                nc.sync.dma_start(qb_sb[:], qb)
                nc.sync.dma_start(kb_sb[:], kb)
            pend1 = None
            for sc in range(NSC):
                emit_proj_chunk(xin, sc, mid_cb=pend1)
                pend1 = None
                if sc == 0:
                    nc.sync.dma_start(wd_sb[:], wd)  # dense-weight prefetch
                last = sc == NSC - 1
                pend = None
                for h in range(HPG):
                    if h % 2 == 0:
                        dch = small.tile([2, QC], F32, tag="dch")
                    fast_h = last and h == HPG - 1
                    cur_dcp = emit_attn(h, sc, dch, skip_dma=fast_h)
                    if h == 2 and pend is not None:
                        # ch0's chain was deferred past h2's attention so its
                        # reciprocal never stalls the vector queue waiting on
                        # the den bounce DMA
                        emit_den_chain(0, sc, pend)
                        pend = None
                    if sc in (1, 2):
                        emit_dense_st(4 * (sc - 1) + h)
                    elif last and h < HPG - 1:
                        emit_dense_st(8 + h)
                        if h == HPG - 2:
                            emit_dense_st(11)
                    if h % 2 == 1:
                        fast = last and h == HPG - 1
                        if h == 1:
                            pend = dch
                            continue
                        if fast:
                            # keep the PE p-state up while the final
                            # reciprocal chain drains
                            wp = psc.tile([128, EGRP, QC], F32, tag="psc")
                            for _ in range(10):
                                nc.tensor.matmul(
                                    wp[:, 0, :],
                                    lhsT=warm_sb[:, 0:128],
                                    rhs=warm_sb[:],
                                    start=True,
                                    stop=True,
                                    skip_group_check=True,
                                )
                        odd_dcp = cur_dcp
                        if not last and h == HPG - 1:
                            pend1 = (
                                lambda ch=h // 2, j=sc, d=dch:
                                emit_den_chain(ch, j, d)
                            )
                        else:
                            emit_den_chain(
                                h // 2, sc, dch, odd_dcp=odd_dcp, fast=fast
                            )
                    if last and h == HPG - 1:
                        for st in (12, 13, 14, 15):
                            emit_dense_st(st, tail=True)

    nc.compile()
    return nc


def _classify_mask(mask):
    """Classify 128(k) x 128(q) score blocks from the actual mask contents.

    Returns (ktiles, mk_arr, binary):
      ktiles[j]: list of (t, lo, tri) per computed k-tile for q-chunk j:
        lo: first kept 128-col block within the 512-wide q-chunk (cols
            [0, lo*128) are fully masked and simply never computed/read)
        tri: [(col_block, uid), ...] 128-col blocks needing masking
      mk_arr: [128, NU, 128] bf16 multiplicative factors exp(-1e9*m/8)
      binary: per-uid flag, True when the factor tile is 0/1-valued
    """
    m2 = np.asarray(mask, dtype=np.float32).reshape(S, S)
    F = np.exp(m2 * np.float32(-1.25e8))  # exp(-1e9*m/8); 0/1 masks -> 0/1
    if (F.max(axis=1) == 0.0).any():
        raise RuntimeError("mask has fully-masked rows; unsupported")
    blocks = F.reshape(NKT, 128, NKT, 128)  # [qi, qr, t, kr]
    kept = (blocks == 1.0).all(axis=(1, 3))  # [qi, t]
    skip = (blocks == 0.0).all(axis=(1, 3))

    NB = QC // 128  # 128-col blocks per q-chunk
    ktiles = []
    uniq = {}
    mk_tiles = []

    def factor_uid(qi, t):
        fb = np.ascontiguousarray(
            F[qi * 128 : (qi + 1) * 128, t * KT : (t + 1) * KT].T
        ).astype(NPBF16)
        key = fb.tobytes()
        if key not in uniq:
            uniq[key] = len(mk_tiles)
            mk_tiles.append(fb)
        return uniq[key]

    for j in range(NQC):
        qis = list(range(j * NB, (j + 1) * NB))
        tl = []
        for t in range(NKT):
            stats = [
                "k" if kept[qi, t] else ("s" if skip[qi, t] else "m")
                for qi in qis
            ]
            if all(s == "s" for s in stats):
                continue
            lo = next(i for i, s in enumerate(stats) if s != "s")
            tri = []
            for i in range(lo, NB):
                if stats[i] == "k":
                    continue
                # mixed OR interior skip (multiply by its factor / zeros)
                tri.append((i, factor_uid(qis[i], t)))
            tl.append((t, lo, tri))
        if not tl:
            raise RuntimeError("q-chunk with no kept k-tiles; unsupported")
        # the first computed tile must span the full chunk (av 'start' MM)
        if tl[0][1] != 0:
            t0, _, tri0 = tl[0]
            tri0 = [(i, u) for i, u in tri0]
            have = {i for i, _ in tri0}
            for i in range(tl[0][1]):
                if i not in have:
                    tri0.append((i, factor_uid(qis[i], t0)))
            tl[0] = (t0, 0, sorted(tri0))
        ktiles.append(tl)
    if not mk_tiles:
        mk_tiles.append(np.ones((128, KT), dtype=NPBF16))
    binary = tuple(
        bool(np.isin(t.astype(np.float32), [0.0, 1.0]).all()) for t in mk_tiles
    )
    mk_arr = np.ascontiguousarray(np.stack(mk_tiles, axis=0).transpose(1, 0, 2))
    # additive variant: NEG where factor==0, 0 where factor==1
    am_arr = np.where(
        mk_arr.astype(np.float32) == 0.0, np.float32(NEG), np.float32(0.0)
    ).astype(NPBF16)
    return ktiles, mk_arr, am_arr, binary


def _xt_prep(x):
    """[S, D] f32 -> [NSC, 128, DK, SC] bf16, d-major, contiguous S-quarters."""
    xt = x.T.astype(NPBF16)  # [D, S]
    a = xt.reshape(DK, 128, NSC, SC).transpose(2, 1, 0, 3)
    return np.ascontiguousarray(a)


def kernel(v, k, q, mask, wq_w, wq_b, wk_w, wk_b, wv_w, wv_b, dense_w, dense_b):
    global LAST_RESULT
    v = np.asarray(v, dtype=np.float32)
    k = np.asarray(k, dtype=np.float32)
    q = np.asarray(q, dtype=np.float32)
    mask = np.asarray(mask, dtype=np.float32)
    wq_w = np.asarray(wq_w, dtype=np.float32)
    wk_w = np.asarray(wk_w, dtype=np.float32)
    wv_w = np.asarray(wv_w, dtype=np.float32)
    dense_w = np.asarray(dense_w, dtype=np.float32)
    wq_b = np.asarray(wq_b, dtype=np.float32)
    wk_b = np.asarray(wk_b, dtype=np.float32)
    wv_b = np.asarray(wv_b, dtype=np.float32)
    dense_b = np.asarray(dense_b, dtype=np.float32)

    ktiles, mk_arr, am_arr, binary = _classify_mask(mask)
    zero_bias = not (np.any(wq_b) or np.any(wk_b))
    key = (
        tuple(tuple((t, lo, tuple(tri)) for t, lo, tri in tl) for tl in ktiles),
        mk_arr.shape[1],
        zero_bias,
        binary,
    )
    if key not in _CACHE:
        _CACHE[key] = _build(ktiles, mk_arr.shape[1], zero_bias, binary)
    nc = _CACHE[key]

    # per-batch inputs (shared by the 4 cores of each batch)
    xq_b = [_xt_prep(q[b]) for b in range(B)]
    xk_b = [_xt_prep(k[b]) for b in range(B)]
    xv_b = [_xt_prep(v[b]) for b in range(B)]

    # per-group weights
    def wslice(w, g):
        ws = w[g * OG : (g + 1) * OG, :].T.astype(NPBF16)  # [D, OG]
        return np.ascontiguousarray(ws.reshape(DK, 128, OG).transpose(1, 0, 2))

    def bslice(b_, g):
        return np.ascontiguousarray(
            b_[g * OG : (g + 1) * OG].astype(np.float32).reshape(2, 128).T
        )

    wq_g = [wslice(wq_w, g) for g in range(GROUPS)]
    wk_g = [wslice(wk_w, g) for g in range(GROUPS)]
    wv_g = [wslice(wv_w, g) for g in range(GROUPS)]
    qb_g = [bslice(wq_b, g) for g in range(GROUPS)]
    kb_g = [bslice(wk_b, g) for g in range(GROUPS)]
    wd_g = []
    for g in range(GROUPS):
        ds = dense_w[:, g * OG : (g + 1) * OG].T.astype(NPBF16)  # [OG, D]
        wd_g.append(np.ascontiguousarray(ds.reshape(2, 128, D_MODEL).transpose(1, 0, 2)))

    id_arr = np.eye(128, dtype=NPBF16)
    # split broadcast selectors (both read reciprocal row 0 of their rhs):
    # slot 0 covers even-head partitions 0:64, slot 1 odd partitions 64:128
    eb_arr = np.zeros((128, 2, 128), dtype=NPBF16)
    eb_arr[0, 0, 0:64] = 1.0
    eb_arr[0, 1, 64:128] = 1.0

    in_maps = []
    for c in range(N_CORES):
        b, g = c // GROUPS, c % GROUPS
        in_maps.append(
            {
                "xq": xq_b[b],
                "xk": xk_b[b],
                "xv": xv_b[b],
                "wq": wq_g[g],
                "wk": wk_g[g],
                "wv": wv_g[g],
                "wd": wd_g[g],
                "qb": qb_g[g],
                "kb": kb_g[g],
                "mk": mk_arr,
                "am": am_arr,
                "ident": id_arr,
                "eb": eb_arr,
            }
        )

    kw = dict(trace=True, **TRACE_KW) if TRACE else {}
    res = run_bass_kernel_spmd(nc, in_maps, core_ids=list(range(N_CORES)), **kw)
    LAST_RESULT = res

    corr = dense_w @ wv_b + dense_b  # v-bias pushed through dense, + dense bias
    out = np.empty((B, S, D_MODEL), dtype=np.float32)
    for b in range(B):
        acc = np.zeros((S, D_MODEL), dtype=np.float32)
        for g in range(GROUPS):
            acc += res.results[b * GROUPS + g]["outp"]
        out[b] = acc + corr
    return out
